# revision 17
# baseline (speedup 1.0000x reference)
"""Trainium2 Bass kernel for nn_CenterLoss (retrieval_knn), v3.

reference semantics (per batch b):
    dist[n, m] = ||pred[b, n] - gt[b, m]||^2           (N=4096, M=512)
    dist1[n] = min_m dist ; dist2[m] = min_n dist
    loss = sum(dist1*obj)/(sum(obj)+1e-6) + sum(dist2*mask)/(sum(mask)+1e-6)

Strategy: data-parallel over batch (16 batches -> 8 cores, 2 each). Per
batch the PE builds the NEGATED distance matrix T = -dist via a K=20
augmented matmul (hi/lo bf16 split for fp32-level precision):
    T[i, j] = sum_k pa[k, i] * ga[k, j]

v3 changes vs the 62us baseline:
  * Row-tiled matmuls: K=20 <= 32, so 4 pred tiles are packed into the 4
    32-row groups of the PE array (tile_position via base_partition) and
    run CONCURRENTLY -> ~4x PE throughput and a warm PE clock.
  * dist1 via fused tensor_scalar+accumulator: one DVE op per tile
    computes out = T*obj (per-partition fp32 scalar) and
    accum_out = rowmax(out), running at the 4x_2p DVE perf mode
    (4 elem/cycle/lane).  obj in {0,1} makes rowmax(obj*T) = obj*rowmax(T)
    (T <= 0), so the objectness mask rides along for free.
  * dist2 via an fp16 pairwise max fold (2x DVE mode) over evicted tiles,
    then PE-transpose + tensor_scalar-accum on PSUM.
  * ACT (scalar engine) evicts PSUM->SBUF fp16 one 4-tile quad at a time.
  * sum(obj) / sum(mask) are computed on host; the device returns
    [s1, s2] partial sums per batch.
"""

import numpy as np

B, N, M = 16, 4096, 512
N_CORES = 8
B_LOC = B // N_CORES        # batches per core
NT = N // 128               # pred tiles per batch (32)
NQ = NT // 4                # quads (4 row-tiled tiles) per batch (8)
GT = M // 128               # gt blocks per batch (4)

_PROGRAM_CACHE = {}


def _install_walrus_ctrl_wait_workaround():
    """The installed walrus rejects multi-wait CTRL (Drain) instructions
    ("Too many sync wait commands"). Split the TileContext end-of-kernel
    drain's sem waits onto individual NOPs (one wait each) on the same
    serial sync engine — semantically equivalent."""
    import concourse.tile as tile
    import concourse.mybir as mybir
    from concourse.vector_clock import ScopedClock

    if getattr(tile.TileContext, "_ctrl_wait_workaround", False):
        return

    def _drain_and_barrier(self, tick_clock, wait_clock):
        nc = self.nc
        drain_inst = nc.sync.drain()
        wait_clock.add_sem_waits(
            drain_inst.ins, ScopedClock({None: tick_clock.global_clock})
        )
        # Move every final wait onto GpSimd (one single-wait NOP each — the
        # walrus limit), then let GpSimd alone clear the semaphores.
        si = drain_inst.ins.sync_info
        if si is not None and si.on_wait:
            waits = list(si.on_wait)
            si.on_wait.clear()
            for w in waits:
                nop_inst = nc.gpsimd.nop()
                nop_inst.ins.sync_info = mybir.SyncInfo(on_wait=[w], on_update=[])

        assert self.sems is not None
        popped = nc._tile_sem_poison_stack.pop()
        assert popped is self._sem_poison
        nc.clear_and_free_semaphores(list(self.sems.allocated().values()))

    tile.TileContext._drain_and_barrier = _drain_and_barrier
    tile.TileContext._ctrl_wait_workaround = True


def _split_multi_waits_json(bir_bytes):
    """The installed walrus accepts at most one sem-wait per instruction.
    Rewrite the serialized BIR: any instruction carrying N>1 waits keeps its
    last wait and gets N-1 single-wait NoOps inserted just before it on the
    same (in-order) engine queue."""
    import orjson

    bir = orjson.loads(bir_bytes)
    counter = [0]
    for fn in bir["functions"]:
        for blk in fn["blocks"]:
            new_insts = []
            for ins in blk["instructions"]:
                si = ins.get("sync_info")
                if si and len(si.get("on_wait") or []) > 1:
                    waits = si["on_wait"]
                    for w in waits[:-1]:
                        counter[0] += 1
                        new_insts.append({
                            "debug": ins.get("debug"),
                            "engine": ins["engine"],
                            "ins": [],
                            "name": f"I-waitsplit-{counter[0]}",
                            "opcode": "NoOp",
                            "outs": [],
                            "sync_info": {"on_update": [], "on_wait": [w]},
                        })
                    si["on_wait"] = [waits[-1]]
                new_insts.append(ins)
            blk["instructions"] = new_insts
    return orjson.dumps(bir)


def _build_program():
    _install_walrus_ctrl_wait_workaround()
    import concourse.bass as bass
    import concourse.tile as tile
    from concourse import mybir
    from concourse.masks import make_identity

    f32 = mybir.dt.float32
    bf16 = mybir.dt.bfloat16
    f16 = mybir.dt.float16
    X = mybir.AxisListType.X
    mx = mybir.AluOpType.max
    mul = mybir.AluOpType.mult
    add = mybir.AluOpType.add

    nc = bass.Bass()
    # pa: row-group packed pred-side aug rows.  partition 32u+k (k<20) holds
    # aug row k of pred tile t=4q+u; free dim is (q, j) with j the pred
    # column inside the tile.
    pa_d = nc.declare_dram_parameter("pa", [B_LOC, 128, NQ * 128], bf16, isOutput=False)
    # ga: gt-side aug rows replicated into all four 32-row groups.
    ga_d = nc.declare_dram_parameter("ga", [B_LOC, 128, M], bf16, isOutput=False)
    obj_d = nc.declare_dram_parameter("obj", [B_LOC, 128, NT], f32, isOutput=False)
    msk_d = nc.declare_dram_parameter("msk", [B_LOC, 128, GT], f32, isOutput=False)
    out_d = nc.declare_dram_parameter("out", [B_LOC * 2], f32, isOutput=True)

    with tile.TileContext(nc) as tc:
        with (
            tc.tile_pool(name="consts", bufs=1) as consts,
            tc.tile_pool(name="inputs", bufs=2) as inputs,
            tc.tile_pool(name="ev", bufs=4) as ev,
            tc.tile_pool(name="fold", bufs=2) as fold,
            tc.tile_pool(name="junk", bufs=2) as junkp,
            tc.tile_pool(name="bat", bufs=2) as bat,
            tc.tile_pool(name="mm", bufs=2, space="PSUM") as mmp,
        ):
            ident = consts.tile([128, 128], f16)
            make_identity(nc, ident[:])
            ones = consts.tile([128, 1], f32)
            nc.vector.memset(ones[:], 1.0)
            pp = consts.tile([128, B_LOC * 2], f32)
            # warm up ACT's Copy table while DMAs are in flight
            warm = consts.tile([1, 2], f32)
            nc.vector.memset(warm[:, 0:1], 0.0)
            nc.scalar.copy(out=warm[:, 1:2], in_=warm[:, 0:1])

            per_batch = []
            for b in range(B_LOC):
                # critical-path inputs (pa/ga of batch 0) on the HWDGE queues;
                # quad 0's weights DMA'd separately so matmuls start early
                pa_sb = inputs.tile([128, NQ * 128], bf16, tag="pa")
                ga_sb = inputs.tile([128, M], bf16, tag="ga")
                if b == 0:
                    nc.sync.dma_start(out=ga_sb[:], in_=ga_d[b])
                    nc.sync.dma_start(out=pa_sb[:, 0:128], in_=pa_d[b][:, 0:128])
                    nc.scalar.dma_start(out=pa_sb[:, 128:], in_=pa_d[b][:, 128:])
                else:
                    nc.gpsimd.dma_start(out=ga_sb[:], in_=ga_d[b])
                    nc.gpsimd.dma_start(out=pa_sb[:], in_=pa_d[b])
                obj_sb = inputs.tile([128, NT], f32, tag="obj")
                nc.gpsimd.dma_start(out=obj_sb[:], in_=obj_d[b])
                msk_sb = inputs.tile([128, GT], f32, tag="msk")
                nc.gpsimd.dma_start(out=msk_sb[:], in_=msk_d[b])

                macc = bat.tile([128, 2, M], f16, tag="macc")
                nc.vector.memset(macc[:], -60000.0)
                d1 = bat.tile([128, NT], f32, tag="d1")
                per_batch.append((macc, d1, obj_sb, msk_sb))

                x4_prev = None
                for q in range(NQ):
                    ps = mmp.tile([128, 4, 512], f32, tag="quad")
                    for u in range(4):
                        nc.tensor.matmul(
                            ps[:, u, :],
                            pa_sb[32 * u : 32 * u + 20, q * 128 : (q + 1) * 128],
                            ga_sb[32 * u : 32 * u + 20, :],
                            start=True,
                            stop=True,
                            tile_position=(32 * u, 0),
                        )
                    # ACT: PSUM fp32 -> SBUF fp16, whole quad in one op
                    x4 = ev.tile([128, 4, 512], f16, tag="x4")
                    nc.scalar.copy(out=x4[:], in_=ps[:])

                    # dist1 tree level 1 for this quad (2x fp16 mode)
                    if q % 2 == 0:
                        h1 = fold.tile([128, 8, 256], f16, tag="h1")
                    nc.vector.tensor_tensor(
                        out=h1[:, (q % 2) * 4 : (q % 2) * 4 + 4, :],
                        in0=x4[:, :, 0:256],
                        in1=x4[:, :, 256:512],
                        op=mx,
                    )

                    if q % 2 == 0:
                        x4_prev = x4
                    else:
                        # dist1 tree levels 2..5 + final reduce for 8 tiles
                        h2 = fold.tile([128, 8, 128], f16, tag="h2")
                        nc.vector.tensor_tensor(
                            out=h2[:], in0=h1[:, :, 0:128], in1=h1[:, :, 128:256], op=mx
                        )
                        h3 = fold.tile([128, 8, 64], f16, tag="h3")
                        nc.vector.tensor_tensor(
                            out=h3[:], in0=h2[:, :, 0:64], in1=h2[:, :, 64:128], op=mx
                        )
                        h4 = fold.tile([128, 8, 32], f16, tag="h4")
                        nc.vector.tensor_tensor(
                            out=h4[:], in0=h3[:, :, 0:32], in1=h3[:, :, 32:64], op=mx
                        )
                        h5 = fold.tile([128, 8, 16], f16, tag="h5")
                        nc.vector.tensor_tensor(
                            out=h5[:], in0=h4[:, :, 0:16], in1=h4[:, :, 16:32], op=mx
                        )
                        nc.vector.tensor_reduce(
                            out=d1[:, (q - 1) * 4 : (q + 1) * 4], in_=h5[:],
                            axis=X, op=mx,
                        )

                        # dist2 fold: pairwise fp16 max tree over the quad pair
                        u1 = fold.tile([128, 4, 512], f16, tag="u1")
                        nc.vector.tensor_tensor(
                            out=u1[:], in0=x4_prev[:], in1=x4[:], op=mx
                        )
                        u2 = fold.tile([128, 2, 512], f16, tag="u2")
                        nc.vector.tensor_tensor(
                            out=u2[:], in0=u1[:, 0:2, :], in1=u1[:, 2:4, :], op=mx
                        )
                        nc.vector.tensor_tensor(
                            out=macc[:], in0=macc[:], in1=u2[:], op=mx
                        )

            # batch tails, deferred so batch 0's tail overlaps batch 1's work
            for b in range(B_LOC):
                macc, d1, obj_sb, msk_sb = per_batch[b]
                # dist2: combine the two macc lanes, transpose 128-blocks into
                # one PSUM quad slot, then per-block rowmax via
                # tensor_scalar accumulator
                mc = fold.tile([128, M], f16, tag="mc")
                nc.vector.tensor_tensor(
                    out=mc[:], in0=macc[:, 0, :], in1=macc[:, 1, :], op=mx
                )
                tp = mmp.tile([128, 4, 1024], f16, tag="quad")
                d2 = bat.tile([128, GT], f32, tag="d2")
                for g in range(GT):
                    nc.tensor.transpose(
                        tp[:, g, 0:128], mc[:, g * 128 : (g + 1) * 128], ident[:]
                    )
                for g in range(GT):
                    jk2 = junkp.tile([128, 128], f16, tag="junk2")
                    nc.vector.tensor_scalar(
                        out=jk2[:],
                        in0=tp[:, g, 0:128],
                        scalar1=1.0,
                        scalar2=None,
                        op0=mul,
                        op1=mx,
                        accum_out=d2[:, g : g + 1],
                    )

                # per-partition partials: pp[:, 2b] = sum(d1*obj),
                # pp[:, 2b+1] = sum(d2 * mask) (both negated dist sums)
                j1 = bat.tile([128, NT], f32, tag="j1")
                nc.vector.tensor_tensor(out=j1[:], in0=d1[:], in1=obj_sb[:], op=mul)
                nc.vector.tensor_scalar(
                    out=j1[:], in0=j1[:], scalar1=1.0, scalar2=None,
                    op0=mul, op1=add, accum_out=pp[:, 2 * b : 2 * b + 1],
                )
                j2 = bat.tile([128, GT], f32, tag="j2")
                nc.vector.tensor_tensor(out=j2[:], in0=d2[:], in1=msk_sb[:], op=mul)
                nc.vector.tensor_scalar(
                    out=j2[:], in0=j2[:], scalar1=1.0, scalar2=None,
                    op0=mul, op1=add, accum_out=pp[:, 2 * b + 1 : 2 * b + 2],
                )

            # cross-partition sum of all partials in one 1-column matmul
            pof = mmp.tile([128, 4, 512], f32, tag="quad")
            po = pof[0 : B_LOC * 2, 0, 0:1]
            nc.tensor.matmul(po, pp[:], ones[:], start=True, stop=True)
            po_sb = consts.tile([B_LOC * 2, 1], f32)
            nc.vector.tensor_copy(out=po_sb[:], in_=po)
            nc.sync.dma_start(out=out_d[:], in_=po_sb[:, 0])

    _orig_to_json_bytes = nc.to_json_bytes
    nc.to_json_bytes = lambda: _split_multi_waits_json(_orig_to_json_bytes())
    return nc


def _get_program():
    if "nc" not in _PROGRAM_CACHE:
        _PROGRAM_CACHE["nc"] = _build_program()
    return _PROGRAM_CACHE["nc"]


def _hi_lo_split(x, bf16):
    hi = x.astype(bf16)
    lo = (x - hi.astype(np.float32)).astype(bf16)
    return hi, lo


def _prep_core_inputs(pred, gt, obj, mask):
    """pred (B_LOC,N,3) gt (B_LOC,M,3) obj (B_LOC,N) int32 mask (B_LOC,M).

    The matmul runs in bf16 with a hi/lo split (K=20): the four hi/lo row
    groups reproduce the fp32 dot products to ~2^-18 at bf16 PE speed."""
    import ml_dtypes
    bf16 = ml_dtypes.bfloat16

    pred = np.asarray(pred, np.float32)
    gt = np.asarray(gt, np.float32)
    pa = np.empty((B_LOC, 5, N), np.float32)
    pa[:, 0:3] = -pred.transpose(0, 2, 1)
    pa[:, 3] = -np.square(pred).sum(-1)
    pa[:, 4] = -1.0
    ga = np.empty((B_LOC, 5, M), np.float32)
    ga[:, 0:3] = -2.0 * gt.transpose(0, 2, 1)
    ga[:, 3] = 1.0
    ga[:, 4] = np.square(gt).sum(-1)

    pa_hi, pa_lo = _hi_lo_split(pa, bf16)
    ga_hi, ga_lo = _hi_lo_split(ga, bf16)
    pa20 = np.concatenate([pa_hi, pa_hi, pa_lo, pa_lo], axis=1)  # [B_LOC,20,N]
    ga20 = np.concatenate([ga_hi, ga_lo, ga_hi, ga_lo], axis=1)  # [B_LOC,20,M]

    # pack pa into 32-row groups: partition 32u+k holds aug row k of tile
    # t=4q+u, free dim (q, j)
    pa_r = pa20.reshape(B_LOC, 20, NQ, 4, 128)
    pa_packed = np.zeros((B_LOC, 4, 32, NQ, 128), bf16)
    pa_packed[:, :, 0:20] = pa_r.transpose(0, 3, 1, 2, 4)
    pa_packed = pa_packed.reshape(B_LOC, 128, NQ * 128)

    # replicate ga into all four row groups
    ga_rep = np.zeros((B_LOC, 4, 32, M), bf16)
    ga_rep[:, :, 0:20] = ga20[:, None, :, :]
    ga_rep = ga_rep.reshape(B_LOC, 128, M)

    ob = np.ascontiguousarray(
        np.asarray(obj, np.float32).reshape(B_LOC, NT, 128).transpose(0, 2, 1)
    )
    mk = np.ascontiguousarray(
        np.asarray(mask, np.float32).reshape(B_LOC, GT, 128).transpose(0, 2, 1)
    )
    return {"pa": pa_packed, "ga": ga_rep, "obj": ob, "msk": mk}


def run(pred_center, center_label, box_label_mask, objectness_label, trace=False):
    """Run the sharded kernel; returns (loss_scalar, BassKernelResults)."""
    from concourse.bass_utils import run_bass_kernel_spmd

    nc = _get_program()
    in_maps = []
    for c in range(N_CORES):
        bs = slice(B_LOC * c, B_LOC * (c + 1))
        in_maps.append(
            _prep_core_inputs(
                pred_center[bs], center_label[bs],
                objectness_label[bs], box_label_mask[bs],
            )
        )
    res = run_bass_kernel_spmd(nc, in_maps, list(range(N_CORES)), trace=trace)
    q = np.stack(
        [res.results[c]["out"].reshape(B_LOC, 2) for c in range(N_CORES)]
    ).astype(np.float64)
    s1 = -q[..., 0].sum()
    s2 = -q[..., 1].sum()
    sum_obj = float(np.asarray(objectness_label, np.float64).sum())
    sum_mask = float(np.asarray(box_label_mask, np.float64).sum())
    loss = s1 / (sum_obj + 1e-6) + s2 / (sum_mask + 1e-6)
    return np.float32(loss), res


def kernel(pred_center, center_label, box_label_mask, objectness_label):
    loss, _ = run(pred_center, center_label, box_label_mask, objectness_label)
    return np.array(loss, dtype=np.float32)


# revision 23
# speedup vs baseline: 1.1592x; 1.1592x over previous
"""Trainium2 Bass kernel for nn_CenterLoss (retrieval_knn), v3.

reference semantics (per batch b):
    dist[n, m] = ||pred[b, n] - gt[b, m]||^2           (N=4096, M=512)
    dist1[n] = min_m dist ; dist2[m] = min_n dist
    loss = sum(dist1*obj)/(sum(obj)+1e-6) + sum(dist2*mask)/(sum(mask)+1e-6)

Strategy: data-parallel over batch (16 batches -> 8 cores, 2 each). Per
batch the PE builds the NEGATED distance matrix T = -dist via a K=20
augmented matmul (hi/lo bf16 split for fp32-level precision):
    T[i, j] = sum_k pa[k, i] * ga[k, j]

v3 changes vs the 62us baseline:
  * Row-tiled matmuls: K=20 <= 32, so 4 pred tiles are packed into the 4
    32-row groups of the PE array (tile_position via base_partition) and
    run CONCURRENTLY -> ~4x PE throughput and a warm PE clock.
  * dist1 via fused tensor_scalar+accumulator: one DVE op per tile
    computes out = T*obj (per-partition fp32 scalar) and
    accum_out = rowmax(out), running at the 4x_2p DVE perf mode
    (4 elem/cycle/lane).  obj in {0,1} makes rowmax(obj*T) = obj*rowmax(T)
    (T <= 0), so the objectness mask rides along for free.
  * dist2 via an fp16 pairwise max fold (2x DVE mode) over evicted tiles,
    then PE-transpose + tensor_scalar-accum on PSUM.
  * ACT (scalar engine) evicts PSUM->SBUF fp16 one 4-tile quad at a time.
  * sum(obj) / sum(mask) are computed on host; the device returns
    [s1, s2] partial sums per batch.
"""

import numpy as np

B, N, M = 16, 4096, 512
N_CORES = 8
B_LOC = B // N_CORES        # batches per core
NT = N // 128               # pred tiles per batch (32)
NQ = NT // 4                # quads (4 row-tiled tiles) per batch (8)
GT = M // 128               # gt blocks per batch (4)

_PROGRAM_CACHE = {}


def _install_walrus_ctrl_wait_workaround():
    """The installed walrus rejects multi-wait CTRL (Drain) instructions
    ("Too many sync wait commands"). Split the TileContext end-of-kernel
    drain's sem waits onto individual NOPs (one wait each) on the same
    serial sync engine — semantically equivalent."""
    import concourse.tile as tile
    import concourse.mybir as mybir
    from concourse.vector_clock import ScopedClock

    if getattr(tile.TileContext, "_ctrl_wait_workaround", False):
        return

    def _drain_and_barrier(self, tick_clock, wait_clock):
        nc = self.nc
        drain_inst = nc.sync.drain()
        wait_clock.add_sem_waits(
            drain_inst.ins, ScopedClock({None: tick_clock.global_clock})
        )
        # Move every final wait onto GpSimd (one single-wait NOP each — the
        # walrus limit), then let GpSimd alone clear the semaphores.
        si = drain_inst.ins.sync_info
        if si is not None and si.on_wait:
            waits = list(si.on_wait)
            si.on_wait.clear()
            for w in waits:
                nop_inst = nc.gpsimd.nop()
                nop_inst.ins.sync_info = mybir.SyncInfo(on_wait=[w], on_update=[])

        assert self.sems is not None
        popped = nc._tile_sem_poison_stack.pop()
        assert popped is self._sem_poison
        nc.clear_and_free_semaphores(list(self.sems.allocated().values()))

    tile.TileContext._drain_and_barrier = _drain_and_barrier
    tile.TileContext._ctrl_wait_workaround = True


def _split_multi_waits_json(bir_bytes):
    """The installed walrus accepts at most one sem-wait per instruction.
    Rewrite the serialized BIR: any instruction carrying N>1 waits keeps its
    last wait and gets N-1 single-wait NoOps inserted just before it on the
    same (in-order) engine queue."""
    import orjson

    bir = orjson.loads(bir_bytes)
    counter = [0]
    for fn in bir["functions"]:
        for blk in fn["blocks"]:
            new_insts = []
            for ins in blk["instructions"]:
                si = ins.get("sync_info")
                if si and len(si.get("on_wait") or []) > 1:
                    waits = si["on_wait"]
                    for w in waits[:-1]:
                        counter[0] += 1
                        new_insts.append({
                            "debug": ins.get("debug"),
                            "engine": ins["engine"],
                            "ins": [],
                            "name": f"I-waitsplit-{counter[0]}",
                            "opcode": "NoOp",
                            "outs": [],
                            "sync_info": {"on_update": [], "on_wait": [w]},
                        })
                    si["on_wait"] = [waits[-1]]
                new_insts.append(ins)
            blk["instructions"] = new_insts
    return orjson.dumps(bir)


def _build_program(t1):
    """t1: number of leading pred tiles (multiple of 4) that carry all
    obj=1 preds — dist1 trees run only on those; dist2 uses all tiles."""
    _install_walrus_ctrl_wait_workaround()
    import concourse.bass as bass
    import concourse.tile as tile
    from concourse import mybir
    from concourse.masks import make_identity

    f32 = mybir.dt.float32
    bf16 = mybir.dt.bfloat16
    f16 = mybir.dt.float16
    X = mybir.AxisListType.X
    mx = mybir.AluOpType.max
    mul = mybir.AluOpType.mult
    add = mybir.AluOpType.add

    nc = bass.Bass()
    # pa: row-group packed pred-side aug rows.  partition 32u+k (k<20) holds
    # aug row k of pred tile t=4q+u; free dim is (q, j) with j the pred
    # column inside the tile.
    pa_d = nc.declare_dram_parameter("pa", [B_LOC, 128, NQ * 128], bf16, isOutput=False)
    # ga: gt-side aug rows replicated into all four 32-row groups.
    ga_d = nc.declare_dram_parameter("ga", [B_LOC, 128, M], bf16, isOutput=False)
    obj_d = nc.declare_dram_parameter("obj", [B_LOC, 128, NT], f32, isOutput=False)
    msk_d = nc.declare_dram_parameter("msk", [B_LOC, 128, GT], f32, isOutput=False)
    out_d = nc.declare_dram_parameter("out", [B_LOC * 2], f32, isOutput=True)

    with tile.TileContext(nc) as tc:
        with (
            tc.tile_pool(name="consts", bufs=1) as consts,
            tc.tile_pool(name="inputs", bufs=2) as inputs,
            tc.tile_pool(name="ev", bufs=4) as ev,
            tc.tile_pool(name="fold", bufs=2) as fold,
            tc.tile_pool(name="junk", bufs=2) as junkp,
            tc.tile_pool(name="bat", bufs=2) as bat,
            tc.tile_pool(name="mm", bufs=2, space="PSUM") as mmp,
        ):
            ident = consts.tile([128, 128], f16)
            make_identity(nc, ident[:])
            ones = consts.tile([128, 1], f32)
            nc.vector.memset(ones[:], 1.0)
            pp = consts.tile([128, B_LOC * 2], f32)
            # warm up ACT's Copy table while DMAs are in flight
            warm = consts.tile([1, 2], f32)
            nc.vector.memset(warm[:, 0:1], 0.0)
            nc.scalar.copy(out=warm[:, 1:2], in_=warm[:, 0:1])

            per_batch = []
            for b in range(B_LOC):
                # critical-path inputs (pa/ga of batch 0) on the HWDGE queues;
                # quad 0's weights DMA'd separately so matmuls start early
                pa_sb = inputs.tile([128, NQ * 128], bf16, tag="pa")
                ga_sb = inputs.tile([128, M], bf16, tag="ga")
                if b == 0:
                    nc.sync.dma_start(out=ga_sb[:], in_=ga_d[b])
                    nc.sync.dma_start(out=pa_sb[:, 0:128], in_=pa_d[b][:, 0:128])
                    nc.scalar.dma_start(out=pa_sb[:, 128:], in_=pa_d[b][:, 128:])
                else:
                    nc.gpsimd.dma_start(out=ga_sb[:], in_=ga_d[b])
                    nc.gpsimd.dma_start(out=pa_sb[:], in_=pa_d[b])
                obj_sb = inputs.tile([128, NT], f32, tag="obj")
                nc.gpsimd.dma_start(out=obj_sb[:], in_=obj_d[b])
                msk_sb = inputs.tile([128, GT], f32, tag="msk")
                nc.gpsimd.dma_start(out=msk_sb[:], in_=msk_d[b])

                macc = bat.tile([128, 2, M], f16, tag="macc")
                d1 = bat.tile([128, NT], f32, tag="d1")
                nc.vector.memset(d1[:], 0.0)
                per_batch.append((macc, d1, obj_sb, msk_sb))

                x4_prev = None
                for q in range(NQ):
                    ps = mmp.tile([128, 4, 512], f32, tag="quad")
                    for u in range(4):
                        nc.tensor.matmul(
                            ps[:, u, :],
                            pa_sb[32 * u : 32 * u + 20, q * 128 : (q + 1) * 128],
                            ga_sb[32 * u : 32 * u + 20, :],
                            start=True,
                            stop=True,
                            tile_position=(32 * u, 0),
                        )
                    # ACT: PSUM fp32 -> SBUF fp16, whole quad in one op
                    x4 = ev.tile([128, 4, 512], f16, tag="x4")
                    nc.scalar.copy(out=x4[:], in_=ps[:])

                    # dist1 trees only on the first t1 (obj-sorted) tiles
                    wide = 4 * q + 4 <= t1 and (q % 2 == 1 or 4 * (q + 1) + 4 <= t1)
                    if wide:
                        # 8-wide tree shared across the quad pair
                        if q % 2 == 0:
                            h1 = fold.tile([128, 8, 256], f16, tag="h1")
                        nc.vector.tensor_tensor(
                            out=h1[:, (q % 2) * 4 : (q % 2) * 4 + 4, :],
                            in0=x4[:, :, 0:256],
                            in1=x4[:, :, 256:512],
                            op=mx,
                        )
                        if q % 2 == 1:
                            h2 = fold.tile([128, 8, 128], f16, tag="h2")
                            nc.vector.tensor_tensor(
                                out=h2[:], in0=h1[:, :, 0:128], in1=h1[:, :, 128:256], op=mx
                            )
                            h3 = fold.tile([128, 8, 64], f16, tag="h3")
                            nc.vector.tensor_tensor(
                                out=h3[:], in0=h2[:, :, 0:64], in1=h2[:, :, 64:128], op=mx
                            )
                            h4 = fold.tile([128, 8, 32], f16, tag="h4")
                            nc.vector.tensor_tensor(
                                out=h4[:], in0=h3[:, :, 0:32], in1=h3[:, :, 32:64], op=mx
                            )
                            h5 = fold.tile([128, 8, 16], f16, tag="h5")
                            nc.vector.tensor_tensor(
                                out=h5[:], in0=h4[:, :, 0:16], in1=h4[:, :, 16:32], op=mx
                            )
                            nc.vector.tensor_reduce(
                                out=d1[:, (q - 1) * 4 : (q + 1) * 4], in_=h5[:],
                                axis=X, op=mx,
                            )
                    elif 4 * q + 4 <= t1:
                        # lone quad at the obj boundary: 4-wide tree
                        g1 = fold.tile([128, 4, 256], f16, tag="g1")
                        nc.vector.tensor_tensor(
                            out=g1[:], in0=x4[:, :, 0:256], in1=x4[:, :, 256:512], op=mx
                        )
                        g2 = fold.tile([128, 4, 128], f16, tag="g2")
                        nc.vector.tensor_tensor(
                            out=g2[:], in0=g1[:, :, 0:128], in1=g1[:, :, 128:256], op=mx
                        )
                        g3 = fold.tile([128, 4, 64], f16, tag="g3")
                        nc.vector.tensor_tensor(
                            out=g3[:], in0=g2[:, :, 0:64], in1=g2[:, :, 64:128], op=mx
                        )
                        g4 = fold.tile([128, 4, 32], f16, tag="g4")
                        nc.vector.tensor_tensor(
                            out=g4[:], in0=g3[:, :, 0:32], in1=g3[:, :, 32:64], op=mx
                        )
                        g5 = fold.tile([128, 4, 16], f16, tag="g5")
                        nc.vector.tensor_tensor(
                            out=g5[:], in0=g4[:, :, 0:16], in1=g4[:, :, 16:32], op=mx
                        )
                        nc.vector.tensor_reduce(
                            out=d1[:, 4 * q : 4 * q + 4], in_=g5[:], axis=X, op=mx,
                        )

                    if q % 2 == 0:
                        x4_prev = x4
                    else:
                        # dist2 fold: pairwise fp16 max tree over the quad pair;
                        # the first pair's u2 doubles as the macc accumulator
                        u1 = fold.tile([128, 4, 512], f16, tag="u1")
                        nc.vector.tensor_tensor(
                            out=u1[:], in0=x4_prev[:], in1=x4[:], op=mx
                        )
                        if q == 1:
                            nc.vector.tensor_tensor(
                                out=macc[:], in0=u1[:, 0:2, :], in1=u1[:, 2:4, :], op=mx
                            )
                        else:
                            u2 = fold.tile([128, 2, 512], f16, tag="u2")
                            nc.vector.tensor_tensor(
                                out=u2[:], in0=u1[:, 0:2, :], in1=u1[:, 2:4, :], op=mx
                            )
                            nc.vector.tensor_tensor(
                                out=macc[:], in0=macc[:], in1=u2[:], op=mx
                            )

            # batch tails, deferred so batch 0's tail overlaps batch 1's work
            for b in range(B_LOC):
                macc, d1, obj_sb, msk_sb = per_batch[b]
                # dist2: combine the two macc lanes, transpose 128-blocks into
                # one PSUM quad slot, then per-block rowmax via
                # tensor_scalar accumulator
                mc = fold.tile([128, M], f16, tag="mc")
                nc.vector.tensor_tensor(
                    out=mc[:], in0=macc[:, 0, :], in1=macc[:, 1, :], op=mx
                )
                tp = mmp.tile([128, 4, 1024], f16, tag="quad")
                d2 = bat.tile([128, GT], f32, tag="d2")
                for g in range(GT):
                    nc.tensor.transpose(
                        tp[:, g, 0:128], mc[:, g * 128 : (g + 1) * 128], ident[:]
                    )
                for g in range(GT):
                    jk2 = junkp.tile([128, 128], f16, tag="junk2")
                    nc.vector.tensor_scalar(
                        out=jk2[:],
                        in0=tp[:, g, 0:128],
                        scalar1=1.0,
                        scalar2=None,
                        op0=mul,
                        op1=mx,
                        accum_out=d2[:, g : g + 1],
                    )

                # per-partition partials: pp[:, 2b] = sum(d1*obj),
                # pp[:, 2b+1] = sum(d2 * mask) (both negated dist sums)
                j1 = bat.tile([128, NT], f32, tag="j1")
                nc.vector.tensor_tensor(out=j1[:], in0=d1[:], in1=obj_sb[:], op=mul)
                nc.vector.tensor_scalar(
                    out=j1[:], in0=j1[:], scalar1=1.0, scalar2=None,
                    op0=mul, op1=add, accum_out=pp[:, 2 * b : 2 * b + 1],
                )
                j2 = bat.tile([128, GT], f32, tag="j2")
                nc.vector.tensor_tensor(out=j2[:], in0=d2[:], in1=msk_sb[:], op=mul)
                nc.vector.tensor_scalar(
                    out=j2[:], in0=j2[:], scalar1=1.0, scalar2=None,
                    op0=mul, op1=add, accum_out=pp[:, 2 * b + 1 : 2 * b + 2],
                )

            # cross-partition sum of all partials in one 1-column matmul
            pof = mmp.tile([128, 4, 512], f32, tag="quad")
            po = pof[0 : B_LOC * 2, 0, 0:1]
            nc.tensor.matmul(po, pp[:], ones[:], start=True, stop=True)
            po_sb = consts.tile([B_LOC * 2, 1], f32)
            nc.vector.tensor_copy(out=po_sb[:], in_=po)
            nc.sync.dma_start(out=out_d[:], in_=po_sb[:, 0])

    _orig_to_json_bytes = nc.to_json_bytes
    nc.to_json_bytes = lambda: _split_multi_waits_json(_orig_to_json_bytes())
    return nc


def _get_program(t1):
    if t1 not in _PROGRAM_CACHE:
        _PROGRAM_CACHE[t1] = _build_program(t1)
    return _PROGRAM_CACHE[t1]


def _hi_lo_split(x, bf16):
    hi = x.astype(bf16)
    lo = (x - hi.astype(np.float32)).astype(bf16)
    return hi, lo


def _prep_core_inputs(pred, gt, obj, mask):
    """pred (B_LOC,N,3) gt (B_LOC,M,3) obj (B_LOC,N) int32 mask (B_LOC,M).

    The matmul runs in bf16 with a hi/lo split (K=20): the four hi/lo row
    groups reproduce the fp32 dot products to ~2^-18 at bf16 PE speed."""
    import ml_dtypes
    bf16 = ml_dtypes.bfloat16

    pred = np.asarray(pred, np.float32)
    gt = np.asarray(gt, np.float32)
    obj = np.asarray(obj, np.float32)
    # sort preds so all obj=1 rows land in the leading tiles: dist1 trees
    # then only run on those tiles (dist2 is order-invariant)
    pred = pred.copy()
    obj = obj.copy()
    for b in range(B_LOC):
        perm = np.argsort(-obj[b], kind="stable")
        pred[b] = pred[b][perm]
        obj[b] = obj[b][perm]
    pa = np.empty((B_LOC, 5, N), np.float32)
    pa[:, 0:3] = -pred.transpose(0, 2, 1)
    pa[:, 3] = -np.square(pred).sum(-1)
    pa[:, 4] = -1.0
    ga = np.empty((B_LOC, 5, M), np.float32)
    ga[:, 0:3] = -2.0 * gt.transpose(0, 2, 1)
    ga[:, 3] = 1.0
    ga[:, 4] = np.square(gt).sum(-1)

    pa_hi, pa_lo = _hi_lo_split(pa, bf16)
    ga_hi, ga_lo = _hi_lo_split(ga, bf16)
    pa20 = np.concatenate([pa_hi, pa_hi, pa_lo, pa_lo], axis=1)  # [B_LOC,20,N]
    ga20 = np.concatenate([ga_hi, ga_lo, ga_hi, ga_lo], axis=1)  # [B_LOC,20,M]

    # pack pa into 32-row groups: partition 32u+k holds aug row k of tile
    # t=4q+u, free dim (q, j)
    pa_r = pa20.reshape(B_LOC, 20, NQ, 4, 128)
    pa_packed = np.zeros((B_LOC, 4, 32, NQ, 128), bf16)
    pa_packed[:, :, 0:20] = pa_r.transpose(0, 3, 1, 2, 4)
    pa_packed = pa_packed.reshape(B_LOC, 128, NQ * 128)

    # replicate ga into all four row groups
    ga_rep = np.zeros((B_LOC, 4, 32, M), bf16)
    ga_rep[:, :, 0:20] = ga20[:, None, :, :]
    ga_rep = ga_rep.reshape(B_LOC, 128, M)

    ob = np.ascontiguousarray(
        np.asarray(obj, np.float32).reshape(B_LOC, NT, 128).transpose(0, 2, 1)
    )
    mk = np.ascontiguousarray(
        np.asarray(mask, np.float32).reshape(B_LOC, GT, 128).transpose(0, 2, 1)
    )
    return {"pa": pa_packed, "ga": ga_rep, "obj": ob, "msk": mk}


def run(pred_center, center_label, box_label_mask, objectness_label, trace=False):
    """Run the sharded kernel; returns (loss_scalar, BassKernelResults)."""
    from concourse.bass_utils import run_bass_kernel_spmd

    cnt = np.asarray(objectness_label, np.int64).reshape(B, N).sum(axis=1)
    t1 = int(-(-int(cnt.max()) // 128))         # tiles needed for obj=1 preds
    t1 = min(NT, max(4, -(-t1 // 4) * 4))       # round up to a quad multiple
    nc = _get_program(t1)
    in_maps = []
    for c in range(N_CORES):
        bs = slice(B_LOC * c, B_LOC * (c + 1))
        in_maps.append(
            _prep_core_inputs(
                pred_center[bs], center_label[bs],
                objectness_label[bs], box_label_mask[bs],
            )
        )
    res = run_bass_kernel_spmd(nc, in_maps, list(range(N_CORES)), trace=trace)
    q = np.stack(
        [res.results[c]["out"].reshape(B_LOC, 2) for c in range(N_CORES)]
    ).astype(np.float64)
    s1 = -q[..., 0].sum()
    s2 = -q[..., 1].sum()
    sum_obj = float(np.asarray(objectness_label, np.float64).sum())
    sum_mask = float(np.asarray(box_label_mask, np.float64).sum())
    loss = s1 / (sum_obj + 1e-6) + s2 / (sum_mask + 1e-6)
    return np.float32(loss), res


def kernel(pred_center, center_label, box_label_mask, objectness_label):
    loss, _ = run(pred_center, center_label, box_label_mask, objectness_label)
    return np.array(loss, dtype=np.float32)


# revision 27
# speedup vs baseline: 1.1680x; 1.0076x over previous
"""Trainium2 Bass kernel for nn_CenterLoss (retrieval_knn), v3.

reference semantics (per batch b):
    dist[n, m] = ||pred[b, n] - gt[b, m]||^2           (N=4096, M=512)
    dist1[n] = min_m dist ; dist2[m] = min_n dist
    loss = sum(dist1*obj)/(sum(obj)+1e-6) + sum(dist2*mask)/(sum(mask)+1e-6)

Strategy: data-parallel over batch (16 batches -> 8 cores, 2 each). Per
batch the PE builds the NEGATED distance matrix T = -dist via a K=20
augmented matmul (hi/lo bf16 split for fp32-level precision):
    T[i, j] = sum_k pa[k, i] * ga[k, j]

v3 changes vs the 62us baseline:
  * Row-tiled matmuls: K=20 <= 32, so 4 pred tiles are packed into the 4
    32-row groups of the PE array (tile_position via base_partition) and
    run CONCURRENTLY -> ~4x PE throughput and a warm PE clock.
  * dist1 via fused tensor_scalar+accumulator: one DVE op per tile
    computes out = T*obj (per-partition fp32 scalar) and
    accum_out = rowmax(out), running at the 4x_2p DVE perf mode
    (4 elem/cycle/lane).  obj in {0,1} makes rowmax(obj*T) = obj*rowmax(T)
    (T <= 0), so the objectness mask rides along for free.
  * dist2 via an fp16 pairwise max fold (2x DVE mode) over evicted tiles,
    then PE-transpose + tensor_scalar-accum on PSUM.
  * ACT (scalar engine) evicts PSUM->SBUF fp16 one 4-tile quad at a time.
  * sum(obj) / sum(mask) are computed on host; the device returns
    [s1, s2] partial sums per batch.
"""

import numpy as np

B, N, M = 16, 4096, 512
N_CORES = 8
B_LOC = B // N_CORES        # batches per core
NT = N // 128               # pred tiles per batch (32)
NQ = NT // 4                # quads (4 row-tiled tiles) per batch (8)
GT = M // 128               # gt blocks per batch (4)

_PROGRAM_CACHE = {}


def _install_walrus_ctrl_wait_workaround():
    """The installed walrus rejects multi-wait CTRL (Drain) instructions
    ("Too many sync wait commands"). Split the TileContext end-of-kernel
    drain's sem waits onto individual NOPs (one wait each) on the same
    serial sync engine — semantically equivalent."""
    import concourse.tile as tile
    import concourse.mybir as mybir
    from concourse.vector_clock import ScopedClock

    if getattr(tile.TileContext, "_ctrl_wait_workaround", False):
        return

    def _drain_and_barrier(self, tick_clock, wait_clock):
        nc = self.nc
        drain_inst = nc.sync.drain()
        wait_clock.add_sem_waits(
            drain_inst.ins, ScopedClock({None: tick_clock.global_clock})
        )
        # Move every final wait onto GpSimd (one single-wait NOP each — the
        # walrus limit), then let GpSimd alone clear the semaphores.
        si = drain_inst.ins.sync_info
        if si is not None and si.on_wait:
            waits = list(si.on_wait)
            si.on_wait.clear()
            for w in waits:
                nop_inst = nc.gpsimd.nop()
                nop_inst.ins.sync_info = mybir.SyncInfo(on_wait=[w], on_update=[])

        assert self.sems is not None
        popped = nc._tile_sem_poison_stack.pop()
        assert popped is self._sem_poison
        nc.clear_and_free_semaphores(list(self.sems.allocated().values()))

    tile.TileContext._drain_and_barrier = _drain_and_barrier
    tile.TileContext._ctrl_wait_workaround = True


def _split_multi_waits_json(bir_bytes):
    """The installed walrus accepts at most one sem-wait per instruction.
    Rewrite the serialized BIR: any instruction carrying N>1 waits keeps its
    last wait and gets N-1 single-wait NoOps inserted just before it on the
    same (in-order) engine queue."""
    import orjson

    bir = orjson.loads(bir_bytes)
    counter = [0]
    for fn in bir["functions"]:
        for blk in fn["blocks"]:
            new_insts = []
            for ins in blk["instructions"]:
                si = ins.get("sync_info")
                if si and len(si.get("on_wait") or []) > 1:
                    waits = si["on_wait"]
                    for w in waits[:-1]:
                        counter[0] += 1
                        new_insts.append({
                            "debug": ins.get("debug"),
                            "engine": ins["engine"],
                            "ins": [],
                            "name": f"I-waitsplit-{counter[0]}",
                            "opcode": "NoOp",
                            "outs": [],
                            "sync_info": {"on_update": [], "on_wait": [w]},
                        })
                    si["on_wait"] = [waits[-1]]
                new_insts.append(ins)
            blk["instructions"] = new_insts
    return orjson.dumps(bir)


def _build_program(t1):
    """t1: number of leading pred tiles (multiple of 4) that carry all
    obj=1 preds — dist1 trees run only on those; dist2 uses all tiles."""
    _install_walrus_ctrl_wait_workaround()
    import concourse.bass as bass
    import concourse.tile as tile
    from concourse import mybir
    from concourse.masks import make_identity

    f32 = mybir.dt.float32
    bf16 = mybir.dt.bfloat16
    f16 = mybir.dt.float16
    X = mybir.AxisListType.X
    mx = mybir.AluOpType.max
    mul = mybir.AluOpType.mult
    add = mybir.AluOpType.add

    nc = bass.Bass()
    # pa: row-group packed pred-side aug rows.  partition 32u+k (k<20) holds
    # aug row k of pred tile t=4q+u; free dim is (q, j) with j the pred
    # column inside the tile.
    pa_d = nc.declare_dram_parameter("pa", [B_LOC, 128, NQ * 128], bf16, isOutput=False)
    # ga: gt-side aug rows replicated into all four 32-row groups.
    ga_d = nc.declare_dram_parameter("ga", [B_LOC, 128, M], bf16, isOutput=False)
    obj_d = nc.declare_dram_parameter("obj", [B_LOC, 128, NT], f32, isOutput=False)
    msk_d = nc.declare_dram_parameter("msk", [B_LOC, 128, GT], f32, isOutput=False)
    out_d = nc.declare_dram_parameter("out", [B_LOC * 2], f32, isOutput=True)

    with tile.TileContext(nc) as tc:
        with (
            tc.tile_pool(name="consts", bufs=1) as consts,
            tc.tile_pool(name="inputs", bufs=2) as inputs,
            tc.tile_pool(name="ev", bufs=4) as ev,
            tc.tile_pool(name="fold", bufs=2) as fold,
            tc.tile_pool(name="junk", bufs=2) as junkp,
            tc.tile_pool(name="bat", bufs=2) as bat,
            tc.tile_pool(name="mm", bufs=2, space="PSUM") as mmp,
        ):
            ident = consts.tile([128, 128], f16)
            make_identity(nc, ident[:])
            ones = consts.tile([128, 1], f32)
            nc.vector.memset(ones[:], 1.0)
            pp = consts.tile([128, B_LOC * 2], f32)
            # warm up ACT's Copy table while DMAs are in flight
            warm = consts.tile([1, 2], f32)
            nc.vector.memset(warm[:, 0:1], 0.0)
            nc.scalar.copy(out=warm[:, 1:2], in_=warm[:, 0:1])

            per_batch = []
            for b in range(B_LOC):
                # critical-path inputs (pa/ga of batch 0) on the HWDGE queues;
                # quad 0's weights live in their own tile so the first matmul
                # only waits on a 32KB DMA, not the full pa transfer
                pa0_sb = inputs.tile([128, 128], bf16, tag="pa0")
                pa_sb = inputs.tile([128, (NQ - 1) * 128], bf16, tag="pa")
                ga_sb = inputs.tile([128, M], bf16, tag="ga")
                if b == 0:
                    nc.sync.dma_start(out=ga_sb[:], in_=ga_d[b])
                    nc.sync.dma_start(out=pa0_sb[:], in_=pa_d[b][:, 0:128])
                    nc.scalar.dma_start(out=pa_sb[:], in_=pa_d[b][:, 128:])
                else:
                    nc.gpsimd.dma_start(out=ga_sb[:], in_=ga_d[b])
                    nc.gpsimd.dma_start(out=pa0_sb[:], in_=pa_d[b][:, 0:128])
                    nc.gpsimd.dma_start(out=pa_sb[:], in_=pa_d[b][:, 128:])
                obj_sb = inputs.tile([128, NT], f32, tag="obj")
                nc.gpsimd.dma_start(out=obj_sb[:], in_=obj_d[b])
                msk_sb = inputs.tile([128, GT], f32, tag="msk")
                nc.gpsimd.dma_start(out=msk_sb[:], in_=msk_d[b])

                macc = bat.tile([128, 2, M], f16, tag="macc")
                d1 = bat.tile([128, NT], f32, tag="d1")
                nc.vector.memset(d1[:], 0.0)
                per_batch.append((macc, d1, obj_sb, msk_sb))

                x4_prev = None
                for q in range(NQ):
                    ps = mmp.tile([128, 4, 512], f32, tag="quad")
                    for u in range(4):
                        w = pa0_sb[32 * u : 32 * u + 20, :] if q == 0 else \
                            pa_sb[32 * u : 32 * u + 20, (q - 1) * 128 : q * 128]
                        nc.tensor.matmul(
                            ps[:, u, :],
                            w,
                            ga_sb[32 * u : 32 * u + 20, :],
                            start=True,
                            stop=True,
                            tile_position=(32 * u, 0),
                        )
                    # ACT: PSUM fp32 -> SBUF fp16, whole quad in one op
                    x4 = ev.tile([128, 4, 512], f16, tag="x4")
                    nc.scalar.copy(out=x4[:], in_=ps[:])

                    # dist1 trees only on the first t1 (obj-sorted) tiles
                    wide = 4 * q + 4 <= t1 and (q % 2 == 1 or 4 * (q + 1) + 4 <= t1)
                    if wide:
                        # 8-wide tree shared across the quad pair
                        if q % 2 == 0:
                            h1 = fold.tile([128, 8, 256], f16, tag="h1")
                        nc.vector.tensor_tensor(
                            out=h1[:, (q % 2) * 4 : (q % 2) * 4 + 4, :],
                            in0=x4[:, :, 0:256],
                            in1=x4[:, :, 256:512],
                            op=mx,
                        )
                        if q % 2 == 1:
                            h2 = fold.tile([128, 8, 128], f16, tag="h2")
                            nc.vector.tensor_tensor(
                                out=h2[:], in0=h1[:, :, 0:128], in1=h1[:, :, 128:256], op=mx
                            )
                            h3 = fold.tile([128, 8, 64], f16, tag="h3")
                            nc.vector.tensor_tensor(
                                out=h3[:], in0=h2[:, :, 0:64], in1=h2[:, :, 64:128], op=mx
                            )
                            h4 = fold.tile([128, 8, 32], f16, tag="h4")
                            nc.vector.tensor_tensor(
                                out=h4[:], in0=h3[:, :, 0:32], in1=h3[:, :, 32:64], op=mx
                            )
                            h5 = fold.tile([128, 8, 16], f16, tag="h5")
                            nc.vector.tensor_tensor(
                                out=h5[:], in0=h4[:, :, 0:16], in1=h4[:, :, 16:32], op=mx
                            )
                            nc.vector.tensor_reduce(
                                out=d1[:, (q - 1) * 4 : (q + 1) * 4], in_=h5[:],
                                axis=X, op=mx,
                            )
                    elif 4 * q + 4 <= t1:
                        # lone quad at the obj boundary: 4-wide tree
                        g1 = fold.tile([128, 4, 256], f16, tag="g1")
                        nc.vector.tensor_tensor(
                            out=g1[:], in0=x4[:, :, 0:256], in1=x4[:, :, 256:512], op=mx
                        )
                        g2 = fold.tile([128, 4, 128], f16, tag="g2")
                        nc.vector.tensor_tensor(
                            out=g2[:], in0=g1[:, :, 0:128], in1=g1[:, :, 128:256], op=mx
                        )
                        g3 = fold.tile([128, 4, 64], f16, tag="g3")
                        nc.vector.tensor_tensor(
                            out=g3[:], in0=g2[:, :, 0:64], in1=g2[:, :, 64:128], op=mx
                        )
                        g4 = fold.tile([128, 4, 32], f16, tag="g4")
                        nc.vector.tensor_tensor(
                            out=g4[:], in0=g3[:, :, 0:32], in1=g3[:, :, 32:64], op=mx
                        )
                        g5 = fold.tile([128, 4, 16], f16, tag="g5")
                        nc.vector.tensor_tensor(
                            out=g5[:], in0=g4[:, :, 0:16], in1=g4[:, :, 16:32], op=mx
                        )
                        nc.vector.tensor_reduce(
                            out=d1[:, 4 * q : 4 * q + 4], in_=g5[:], axis=X, op=mx,
                        )

                    if q % 2 == 0:
                        x4_prev = x4
                    else:
                        # dist2 fold: pairwise fp16 max tree over the quad pair;
                        # the first pair's u2 doubles as the macc accumulator
                        u1 = fold.tile([128, 4, 512], f16, tag="u1")
                        nc.vector.tensor_tensor(
                            out=u1[:], in0=x4_prev[:], in1=x4[:], op=mx
                        )
                        if q == 1:
                            nc.vector.tensor_tensor(
                                out=macc[:], in0=u1[:, 0:2, :], in1=u1[:, 2:4, :], op=mx
                            )
                        else:
                            u2 = fold.tile([128, 2, 512], f16, tag="u2")
                            nc.vector.tensor_tensor(
                                out=u2[:], in0=u1[:, 0:2, :], in1=u1[:, 2:4, :], op=mx
                            )
                            nc.vector.tensor_tensor(
                                out=macc[:], in0=macc[:], in1=u2[:], op=mx
                            )

            # batch tails, deferred so batch 0's tail overlaps batch 1's work
            for b in range(B_LOC):
                macc, d1, obj_sb, msk_sb = per_batch[b]
                # dist2: combine the two macc lanes, transpose 128-blocks into
                # one PSUM quad slot, then per-block rowmax via
                # tensor_scalar accumulator
                mc = fold.tile([128, M], f16, tag="mc")
                nc.vector.tensor_tensor(
                    out=mc[:], in0=macc[:, 0, :], in1=macc[:, 1, :], op=mx
                )
                tp = mmp.tile([128, 4, 1024], f16, tag="quad")
                d2 = bat.tile([128, GT], f32, tag="d2")
                for g in range(GT):
                    nc.tensor.transpose(
                        tp[:, g, 0:128], mc[:, g * 128 : (g + 1) * 128], ident[:]
                    )
                for g in range(GT):
                    jk2 = junkp.tile([128, 128], f16, tag="junk2")
                    nc.vector.tensor_scalar(
                        out=jk2[:],
                        in0=tp[:, g, 0:128],
                        scalar1=1.0,
                        scalar2=None,
                        op0=mul,
                        op1=mx,
                        accum_out=d2[:, g : g + 1],
                    )

                # per-partition partials: pp[:, 2b] = sum(d1*obj),
                # pp[:, 2b+1] = sum(d2 * mask) (both negated dist sums)
                j1 = bat.tile([128, NT], f32, tag="j1")
                nc.vector.tensor_tensor(out=j1[:], in0=d1[:], in1=obj_sb[:], op=mul)
                nc.vector.tensor_scalar(
                    out=j1[:], in0=j1[:], scalar1=1.0, scalar2=None,
                    op0=mul, op1=add, accum_out=pp[:, 2 * b : 2 * b + 1],
                )
                j2 = bat.tile([128, GT], f32, tag="j2")
                nc.vector.tensor_tensor(out=j2[:], in0=d2[:], in1=msk_sb[:], op=mul)
                nc.vector.tensor_scalar(
                    out=j2[:], in0=j2[:], scalar1=1.0, scalar2=None,
                    op0=mul, op1=add, accum_out=pp[:, 2 * b + 1 : 2 * b + 2],
                )

            # cross-partition sum of all partials in one 1-column matmul
            pof = mmp.tile([128, 4, 512], f32, tag="quad")
            po = pof[0 : B_LOC * 2, 0, 0:1]
            nc.tensor.matmul(po, pp[:], ones[:], start=True, stop=True)
            po_sb = consts.tile([B_LOC * 2, 1], f32)
            nc.vector.tensor_copy(out=po_sb[:], in_=po)
            nc.sync.dma_start(out=out_d[:], in_=po_sb[:, 0])

    _orig_to_json_bytes = nc.to_json_bytes
    nc.to_json_bytes = lambda: _split_multi_waits_json(_orig_to_json_bytes())
    return nc


def _get_program(t1):
    if t1 not in _PROGRAM_CACHE:
        _PROGRAM_CACHE[t1] = _build_program(t1)
    return _PROGRAM_CACHE[t1]


def _hi_lo_split(x, bf16):
    hi = x.astype(bf16)
    lo = (x - hi.astype(np.float32)).astype(bf16)
    return hi, lo


def _prep_core_inputs(pred, gt, obj, mask):
    """pred (B_LOC,N,3) gt (B_LOC,M,3) obj (B_LOC,N) int32 mask (B_LOC,M).

    The matmul runs in bf16 with a hi/lo split (K=20): the four hi/lo row
    groups reproduce the fp32 dot products to ~2^-18 at bf16 PE speed."""
    import ml_dtypes
    bf16 = ml_dtypes.bfloat16

    pred = np.asarray(pred, np.float32)
    gt = np.asarray(gt, np.float32)
    obj = np.asarray(obj, np.float32)
    # sort preds so all obj=1 rows land in the leading tiles: dist1 trees
    # then only run on those tiles (dist2 is order-invariant)
    pred = pred.copy()
    obj = obj.copy()
    for b in range(B_LOC):
        perm = np.argsort(-obj[b], kind="stable")
        pred[b] = pred[b][perm]
        obj[b] = obj[b][perm]
    pa = np.empty((B_LOC, 5, N), np.float32)
    pa[:, 0:3] = -pred.transpose(0, 2, 1)
    pa[:, 3] = -np.square(pred).sum(-1)
    pa[:, 4] = -1.0
    ga = np.empty((B_LOC, 5, M), np.float32)
    ga[:, 0:3] = -2.0 * gt.transpose(0, 2, 1)
    ga[:, 3] = 1.0
    ga[:, 4] = np.square(gt).sum(-1)

    pa_hi, pa_lo = _hi_lo_split(pa, bf16)
    ga_hi, ga_lo = _hi_lo_split(ga, bf16)
    pa20 = np.concatenate([pa_hi, pa_hi, pa_lo, pa_lo], axis=1)  # [B_LOC,20,N]
    ga20 = np.concatenate([ga_hi, ga_lo, ga_hi, ga_lo], axis=1)  # [B_LOC,20,M]

    # pack pa into 32-row groups: partition 32u+k holds aug row k of tile
    # t=4q+u, free dim (q, j)
    pa_r = pa20.reshape(B_LOC, 20, NQ, 4, 128)
    pa_packed = np.zeros((B_LOC, 4, 32, NQ, 128), bf16)
    pa_packed[:, :, 0:20] = pa_r.transpose(0, 3, 1, 2, 4)
    pa_packed = pa_packed.reshape(B_LOC, 128, NQ * 128)

    # replicate ga into all four row groups
    ga_rep = np.zeros((B_LOC, 4, 32, M), bf16)
    ga_rep[:, :, 0:20] = ga20[:, None, :, :]
    ga_rep = ga_rep.reshape(B_LOC, 128, M)

    ob = np.ascontiguousarray(
        np.asarray(obj, np.float32).reshape(B_LOC, NT, 128).transpose(0, 2, 1)
    )
    mk = np.ascontiguousarray(
        np.asarray(mask, np.float32).reshape(B_LOC, GT, 128).transpose(0, 2, 1)
    )
    return {"pa": pa_packed, "ga": ga_rep, "obj": ob, "msk": mk}


def run(pred_center, center_label, box_label_mask, objectness_label, trace=False):
    """Run the sharded kernel; returns (loss_scalar, BassKernelResults)."""
    from concourse.bass_utils import run_bass_kernel_spmd

    cnt = np.asarray(objectness_label, np.int64).reshape(B, N).sum(axis=1)
    t1 = int(-(-int(cnt.max()) // 128))         # tiles needed for obj=1 preds
    t1 = min(NT, max(4, -(-t1 // 4) * 4))       # round up to a quad multiple
    nc = _get_program(t1)
    in_maps = []
    for c in range(N_CORES):
        bs = slice(B_LOC * c, B_LOC * (c + 1))
        in_maps.append(
            _prep_core_inputs(
                pred_center[bs], center_label[bs],
                objectness_label[bs], box_label_mask[bs],
            )
        )
    res = run_bass_kernel_spmd(nc, in_maps, list(range(N_CORES)), trace=trace)
    q = np.stack(
        [res.results[c]["out"].reshape(B_LOC, 2) for c in range(N_CORES)]
    ).astype(np.float64)
    s1 = -q[..., 0].sum()
    s2 = -q[..., 1].sum()
    sum_obj = float(np.asarray(objectness_label, np.float64).sum())
    sum_mask = float(np.asarray(box_label_mask, np.float64).sum())
    loss = s1 / (sum_obj + 1e-6) + s2 / (sum_mask + 1e-6)
    return np.float32(loss), res


def kernel(pred_center, center_label, box_label_mask, objectness_label):
    loss, _ = run(pred_center, center_label, box_label_mask, objectness_label)
    return np.array(loss, dtype=np.float32)


# revision 30
# speedup vs baseline: 1.1943x; 1.0225x over previous
"""Trainium2 Bass kernel for nn_CenterLoss (retrieval_knn), v3.

reference semantics (per batch b):
    dist[n, m] = ||pred[b, n] - gt[b, m]||^2           (N=4096, M=512)
    dist1[n] = min_m dist ; dist2[m] = min_n dist
    loss = sum(dist1*obj)/(sum(obj)+1e-6) + sum(dist2*mask)/(sum(mask)+1e-6)

Strategy: data-parallel over batch (16 batches -> 8 cores, 2 each). Per
batch the PE builds the NEGATED distance matrix T = -dist via a K=20
augmented matmul (hi/lo bf16 split for fp32-level precision):
    T[i, j] = sum_k pa[k, i] * ga[k, j]

v3 changes vs the 62us baseline:
  * Row-tiled matmuls: K=20 <= 32, so 4 pred tiles are packed into the 4
    32-row groups of the PE array (tile_position via base_partition) and
    run CONCURRENTLY -> ~4x PE throughput and a warm PE clock.
  * dist1 via fused tensor_scalar+accumulator: one DVE op per tile
    computes out = T*obj (per-partition fp32 scalar) and
    accum_out = rowmax(out), running at the 4x_2p DVE perf mode
    (4 elem/cycle/lane).  obj in {0,1} makes rowmax(obj*T) = obj*rowmax(T)
    (T <= 0), so the objectness mask rides along for free.
  * dist2 via an fp16 pairwise max fold (2x DVE mode) over evicted tiles,
    then PE-transpose + tensor_scalar-accum on PSUM.
  * ACT (scalar engine) evicts PSUM->SBUF fp16 one 4-tile quad at a time.
  * sum(obj) / sum(mask) are computed on host; the device returns
    [s1, s2] partial sums per batch.
"""

import numpy as np

B, N, M = 16, 4096, 512
N_CORES = 8
B_LOC = B // N_CORES        # batches per core
NT = N // 128               # pred tiles per batch (32)
NQ = NT // 4                # quads (4 row-tiled tiles) per batch (8)
GT = M // 128               # gt blocks per batch (4)

_PROGRAM_CACHE = {}


def _install_walrus_ctrl_wait_workaround():
    """The installed walrus rejects multi-wait CTRL (Drain) instructions
    ("Too many sync wait commands"). Split the TileContext end-of-kernel
    drain's sem waits onto individual NOPs (one wait each) on the same
    serial sync engine — semantically equivalent."""
    import concourse.tile as tile
    import concourse.mybir as mybir
    from concourse.vector_clock import ScopedClock

    if getattr(tile.TileContext, "_ctrl_wait_workaround", False):
        return

    def _drain_and_barrier(self, tick_clock, wait_clock):
        nc = self.nc
        drain_inst = nc.sync.drain()
        wait_clock.add_sem_waits(
            drain_inst.ins, ScopedClock({None: tick_clock.global_clock})
        )
        # Move every final wait onto GpSimd (one single-wait NOP each — the
        # walrus limit), then let GpSimd alone clear the semaphores.
        si = drain_inst.ins.sync_info
        if si is not None and si.on_wait:
            waits = list(si.on_wait)
            si.on_wait.clear()
            for w in waits:
                nop_inst = nc.gpsimd.nop()
                nop_inst.ins.sync_info = mybir.SyncInfo(on_wait=[w], on_update=[])

        assert self.sems is not None
        popped = nc._tile_sem_poison_stack.pop()
        assert popped is self._sem_poison
        nc.clear_and_free_semaphores(list(self.sems.allocated().values()))

    tile.TileContext._drain_and_barrier = _drain_and_barrier
    tile.TileContext._ctrl_wait_workaround = True


def _split_multi_waits_json(bir_bytes):
    """The installed walrus accepts at most one sem-wait per instruction.
    Rewrite the serialized BIR: any instruction carrying N>1 waits keeps its
    last wait and gets N-1 single-wait NoOps inserted just before it on the
    same (in-order) engine queue."""
    import orjson

    bir = orjson.loads(bir_bytes)
    counter = [0]
    for fn in bir["functions"]:
        for blk in fn["blocks"]:
            new_insts = []
            for ins in blk["instructions"]:
                si = ins.get("sync_info")
                if si and len(si.get("on_wait") or []) > 1:
                    waits = si["on_wait"]
                    for w in waits[:-1]:
                        counter[0] += 1
                        new_insts.append({
                            "debug": ins.get("debug"),
                            "engine": ins["engine"],
                            "ins": [],
                            "name": f"I-waitsplit-{counter[0]}",
                            "opcode": "NoOp",
                            "outs": [],
                            "sync_info": {"on_update": [], "on_wait": [w]},
                        })
                    si["on_wait"] = [waits[-1]]
                new_insts.append(ins)
            blk["instructions"] = new_insts
    return orjson.dumps(bir)


def _build_program(t1):
    """t1: number of leading pred tiles (multiple of 4) that carry all
    obj=1 preds — dist1 trees run only on those; dist2 uses all tiles."""
    _install_walrus_ctrl_wait_workaround()
    import concourse.bass as bass
    import concourse.tile as tile
    from concourse import mybir
    from concourse.masks import make_identity

    f32 = mybir.dt.float32
    bf16 = mybir.dt.bfloat16
    f16 = mybir.dt.float16
    X = mybir.AxisListType.X
    mx = mybir.AluOpType.max
    mul = mybir.AluOpType.mult
    add = mybir.AluOpType.add

    nc = bass.Bass()
    # pa: row-group packed pred-side aug rows.  partition 32u+k (k<20) holds
    # aug row k of pred tile t=4q+u; free dim is (q, j) with j the pred
    # column inside the tile.
    pa_d = nc.declare_dram_parameter("pa", [B_LOC, 128, NQ * 128], bf16, isOutput=False)
    # ga: gt-side aug rows replicated into all four 32-row groups.
    ga_d = nc.declare_dram_parameter("ga", [B_LOC, 128, M], bf16, isOutput=False)
    obj_d = nc.declare_dram_parameter("obj", [B_LOC, 128, NT], f32, isOutput=False)
    msk_d = nc.declare_dram_parameter("msk", [B_LOC, 128, GT], f32, isOutput=False)
    out_d = nc.declare_dram_parameter("out", [B_LOC * 2], f32, isOutput=True)

    with tile.TileContext(nc) as tc:
        with (
            tc.tile_pool(name="consts", bufs=1) as consts,
            tc.tile_pool(name="inputs", bufs=2) as inputs,
            tc.tile_pool(name="ev", bufs=6) as ev,
            tc.tile_pool(name="fold", bufs=3) as fold,
            tc.tile_pool(name="junk", bufs=2) as junkp,
            tc.tile_pool(name="bat", bufs=2) as bat,
            tc.tile_pool(name="mm", bufs=2, space="PSUM") as mmp,
        ):
            ident = consts.tile([128, 128], f16)
            make_identity(nc, ident[:])
            ones = consts.tile([128, 1], f32)
            nc.vector.memset(ones[:], 1.0)
            pp = consts.tile([128, B_LOC * 2], f32)
            # warm up ACT's Copy table while DMAs are in flight
            warm = consts.tile([1, 2], f32)
            nc.vector.memset(warm[:, 0:1], 0.0)
            nc.scalar.copy(out=warm[:, 1:2], in_=warm[:, 0:1])

            per_batch = []
            for b in range(B_LOC):
                # critical-path inputs (pa/ga of batch 0) on the HWDGE queues;
                # quad 0's weights live in their own tile so the first matmul
                # only waits on a 32KB DMA, not the full pa transfer
                pa0_sb = inputs.tile([128, 128], bf16, tag="pa0")
                pa_sb = inputs.tile([128, (NQ - 1) * 128], bf16, tag="pa")
                ga_sb = inputs.tile([128, M], bf16, tag="ga")
                if b == 0:
                    # ga split across both HWDGE queues halves the transfer
                    # time gating the first matmul
                    nc.sync.dma_start(out=ga_sb[0:64, :], in_=ga_d[b][0:64])
                    nc.scalar.dma_start(out=ga_sb[64:128, :], in_=ga_d[b][64:128])
                    nc.sync.dma_start(out=pa0_sb[:], in_=pa_d[b][:, 0:128])
                    nc.scalar.dma_start(out=pa_sb[:], in_=pa_d[b][:, 128:])
                else:
                    nc.gpsimd.dma_start(out=ga_sb[:], in_=ga_d[b])
                    nc.gpsimd.dma_start(out=pa0_sb[:], in_=pa_d[b][:, 0:128])
                    nc.gpsimd.dma_start(out=pa_sb[:], in_=pa_d[b][:, 128:])
                obj_sb = inputs.tile([128, NT], f32, tag="obj")
                nc.gpsimd.dma_start(out=obj_sb[:], in_=obj_d[b])
                msk_sb = inputs.tile([128, GT], f32, tag="msk")
                nc.gpsimd.dma_start(out=msk_sb[:], in_=msk_d[b])

                macc = bat.tile([128, 2, M], f16, tag="macc")
                d1 = bat.tile([128, NT], f32, tag="d1")
                nc.vector.memset(d1[:], 0.0)
                per_batch.append((macc, d1, obj_sb, msk_sb))

                x4_prev = None
                for q in range(NQ):
                    ps = mmp.tile([128, 4, 512], f32, tag="quad")
                    for u in range(4):
                        w = pa0_sb[32 * u : 32 * u + 20, :] if q == 0 else \
                            pa_sb[32 * u : 32 * u + 20, (q - 1) * 128 : q * 128]
                        nc.tensor.matmul(
                            ps[:, u, :],
                            w,
                            ga_sb[32 * u : 32 * u + 20, :],
                            start=True,
                            stop=True,
                            tile_position=(32 * u, 0),
                        )
                    # ACT: PSUM fp32 -> SBUF fp16, whole quad in one op
                    x4 = ev.tile([128, 4, 512], f16, tag="x4")
                    nc.scalar.copy(out=x4[:], in_=ps[:])

                    # dist1 trees only on the first t1 (obj-sorted) tiles
                    wide = 4 * q + 4 <= t1 and (q % 2 == 1 or 4 * (q + 1) + 4 <= t1)
                    if wide:
                        # 8-wide tree shared across the quad pair
                        if q % 2 == 0:
                            h1 = fold.tile([128, 8, 256], f16, tag="h1")
                        nc.vector.tensor_tensor(
                            out=h1[:, (q % 2) * 4 : (q % 2) * 4 + 4, :],
                            in0=x4[:, :, 0:256],
                            in1=x4[:, :, 256:512],
                            op=mx,
                        )
                        if q % 2 == 1:
                            h2 = fold.tile([128, 8, 128], f16, tag="h2")
                            nc.vector.tensor_tensor(
                                out=h2[:], in0=h1[:, :, 0:128], in1=h1[:, :, 128:256], op=mx
                            )
                            h3 = fold.tile([128, 8, 64], f16, tag="h3")
                            nc.vector.tensor_tensor(
                                out=h3[:], in0=h2[:, :, 0:64], in1=h2[:, :, 64:128], op=mx
                            )
                            h4 = fold.tile([128, 8, 32], f16, tag="h4")
                            nc.vector.tensor_tensor(
                                out=h4[:], in0=h3[:, :, 0:32], in1=h3[:, :, 32:64], op=mx
                            )
                            h5 = fold.tile([128, 8, 16], f16, tag="h5")
                            nc.vector.tensor_tensor(
                                out=h5[:], in0=h4[:, :, 0:16], in1=h4[:, :, 16:32], op=mx
                            )
                            nc.vector.tensor_reduce(
                                out=d1[:, (q - 1) * 4 : (q + 1) * 4], in_=h5[:],
                                axis=X, op=mx,
                            )
                    elif 4 * q + 4 <= t1:
                        # lone quad at the obj boundary: 4-wide tree
                        g1 = fold.tile([128, 4, 256], f16, tag="g1")
                        nc.vector.tensor_tensor(
                            out=g1[:], in0=x4[:, :, 0:256], in1=x4[:, :, 256:512], op=mx
                        )
                        g2 = fold.tile([128, 4, 128], f16, tag="g2")
                        nc.vector.tensor_tensor(
                            out=g2[:], in0=g1[:, :, 0:128], in1=g1[:, :, 128:256], op=mx
                        )
                        g3 = fold.tile([128, 4, 64], f16, tag="g3")
                        nc.vector.tensor_tensor(
                            out=g3[:], in0=g2[:, :, 0:64], in1=g2[:, :, 64:128], op=mx
                        )
                        g4 = fold.tile([128, 4, 32], f16, tag="g4")
                        nc.vector.tensor_tensor(
                            out=g4[:], in0=g3[:, :, 0:32], in1=g3[:, :, 32:64], op=mx
                        )
                        g5 = fold.tile([128, 4, 16], f16, tag="g5")
                        nc.vector.tensor_tensor(
                            out=g5[:], in0=g4[:, :, 0:16], in1=g4[:, :, 16:32], op=mx
                        )
                        nc.vector.tensor_reduce(
                            out=d1[:, 4 * q : 4 * q + 4], in_=g5[:], axis=X, op=mx,
                        )

                    if q % 2 == 0:
                        x4_prev = x4
                    else:
                        # dist2 fold: pairwise fp16 max tree over the quad pair;
                        # the first pair's u2 doubles as the macc accumulator
                        u1 = fold.tile([128, 4, 512], f16, tag="u1")
                        nc.vector.tensor_tensor(
                            out=u1[:], in0=x4_prev[:], in1=x4[:], op=mx
                        )
                        if q == 1:
                            nc.vector.tensor_tensor(
                                out=macc[:], in0=u1[:, 0:2, :], in1=u1[:, 2:4, :], op=mx
                            )
                        else:
                            u2 = fold.tile([128, 2, 512], f16, tag="u2")
                            nc.vector.tensor_tensor(
                                out=u2[:], in0=u1[:, 0:2, :], in1=u1[:, 2:4, :], op=mx
                            )
                            nc.vector.tensor_tensor(
                                out=macc[:], in0=macc[:], in1=u2[:], op=mx
                            )

            # batch tails, deferred so batch 0's tail overlaps batch 1's work
            for b in range(B_LOC):
                macc, d1, obj_sb, msk_sb = per_batch[b]
                # dist2: combine the two macc lanes, transpose 128-blocks into
                # one PSUM quad slot, then per-block rowmax via
                # tensor_scalar accumulator
                mc = fold.tile([128, M], f16, tag="mc")
                nc.vector.tensor_tensor(
                    out=mc[:], in0=macc[:, 0, :], in1=macc[:, 1, :], op=mx
                )
                tp = mmp.tile([128, 4, 1024], f16, tag="quad")
                d2 = bat.tile([128, GT], f32, tag="d2")
                for g in range(GT):
                    nc.tensor.transpose(
                        tp[:, g, 0:128], mc[:, g * 128 : (g + 1) * 128], ident[:]
                    )
                nc.vector.tensor_reduce(
                    out=d2[:], in_=tp[:, :, 0:128], axis=X, op=mx
                )

                # per-partition partials: pp[:, 2b] = sum(d1*obj),
                # pp[:, 2b+1] = sum(d2 * mask) (both negated dist sums)
                j1 = bat.tile([128, NT], f32, tag="j1")
                nc.vector.tensor_tensor(out=j1[:], in0=d1[:], in1=obj_sb[:], op=mul)
                nc.vector.tensor_scalar(
                    out=j1[:], in0=j1[:], scalar1=1.0, scalar2=None,
                    op0=mul, op1=add, accum_out=pp[:, 2 * b : 2 * b + 1],
                )
                j2 = bat.tile([128, GT], f32, tag="j2")
                nc.vector.tensor_tensor(out=j2[:], in0=d2[:], in1=msk_sb[:], op=mul)
                nc.vector.tensor_scalar(
                    out=j2[:], in0=j2[:], scalar1=1.0, scalar2=None,
                    op0=mul, op1=add, accum_out=pp[:, 2 * b + 1 : 2 * b + 2],
                )

            # cross-partition sum of all partials in one 1-column matmul
            pof = mmp.tile([128, 4, 512], f32, tag="quad")
            po = pof[0 : B_LOC * 2, 0, 0:1]
            nc.tensor.matmul(po, pp[:], ones[:], start=True, stop=True)
            po_sb = consts.tile([B_LOC * 2, 1], f32)
            nc.vector.tensor_copy(out=po_sb[:], in_=po)
            nc.sync.dma_start(out=out_d[:], in_=po_sb[:, 0])

    _orig_to_json_bytes = nc.to_json_bytes
    nc.to_json_bytes = lambda: _split_multi_waits_json(_orig_to_json_bytes())
    return nc


def _get_program(t1):
    if t1 not in _PROGRAM_CACHE:
        _PROGRAM_CACHE[t1] = _build_program(t1)
    return _PROGRAM_CACHE[t1]


def _hi_lo_split(x, bf16):
    hi = x.astype(bf16)
    lo = (x - hi.astype(np.float32)).astype(bf16)
    return hi, lo


def _prep_core_inputs(pred, gt, obj, mask):
    """pred (B_LOC,N,3) gt (B_LOC,M,3) obj (B_LOC,N) int32 mask (B_LOC,M).

    The matmul runs in bf16 with a hi/lo split (K=20): the four hi/lo row
    groups reproduce the fp32 dot products to ~2^-18 at bf16 PE speed."""
    import ml_dtypes
    bf16 = ml_dtypes.bfloat16

    pred = np.asarray(pred, np.float32)
    gt = np.asarray(gt, np.float32)
    obj = np.asarray(obj, np.float32)
    # sort preds so all obj=1 rows land in the leading tiles: dist1 trees
    # then only run on those tiles (dist2 is order-invariant)
    pred = pred.copy()
    obj = obj.copy()
    for b in range(B_LOC):
        perm = np.argsort(-obj[b], kind="stable")
        pred[b] = pred[b][perm]
        obj[b] = obj[b][perm]
    pa = np.empty((B_LOC, 5, N), np.float32)
    pa[:, 0:3] = -pred.transpose(0, 2, 1)
    pa[:, 3] = -np.square(pred).sum(-1)
    pa[:, 4] = -1.0
    ga = np.empty((B_LOC, 5, M), np.float32)
    ga[:, 0:3] = -2.0 * gt.transpose(0, 2, 1)
    ga[:, 3] = 1.0
    ga[:, 4] = np.square(gt).sum(-1)

    pa_hi, pa_lo = _hi_lo_split(pa, bf16)
    ga_hi, ga_lo = _hi_lo_split(ga, bf16)
    pa20 = np.concatenate([pa_hi, pa_hi, pa_lo, pa_lo], axis=1)  # [B_LOC,20,N]
    ga20 = np.concatenate([ga_hi, ga_lo, ga_hi, ga_lo], axis=1)  # [B_LOC,20,M]

    # pack pa into 32-row groups: partition 32u+k holds aug row k of tile
    # t=4q+u, free dim (q, j)
    pa_r = pa20.reshape(B_LOC, 20, NQ, 4, 128)
    pa_packed = np.zeros((B_LOC, 4, 32, NQ, 128), bf16)
    pa_packed[:, :, 0:20] = pa_r.transpose(0, 3, 1, 2, 4)
    pa_packed = pa_packed.reshape(B_LOC, 128, NQ * 128)

    # replicate ga into all four row groups
    ga_rep = np.zeros((B_LOC, 4, 32, M), bf16)
    ga_rep[:, :, 0:20] = ga20[:, None, :, :]
    ga_rep = ga_rep.reshape(B_LOC, 128, M)

    ob = np.ascontiguousarray(
        np.asarray(obj, np.float32).reshape(B_LOC, NT, 128).transpose(0, 2, 1)
    )
    mk = np.ascontiguousarray(
        np.asarray(mask, np.float32).reshape(B_LOC, GT, 128).transpose(0, 2, 1)
    )
    return {"pa": pa_packed, "ga": ga_rep, "obj": ob, "msk": mk}


def run(pred_center, center_label, box_label_mask, objectness_label, trace=False):
    """Run the sharded kernel; returns (loss_scalar, BassKernelResults)."""
    from concourse.bass_utils import run_bass_kernel_spmd

    cnt = np.asarray(objectness_label, np.int64).reshape(B, N).sum(axis=1)
    t1 = int(-(-int(cnt.max()) // 128))         # tiles needed for obj=1 preds
    t1 = min(NT, max(4, -(-t1 // 4) * 4))       # round up to a quad multiple
    nc = _get_program(t1)
    in_maps = []
    for c in range(N_CORES):
        bs = slice(B_LOC * c, B_LOC * (c + 1))
        in_maps.append(
            _prep_core_inputs(
                pred_center[bs], center_label[bs],
                objectness_label[bs], box_label_mask[bs],
            )
        )
    res = run_bass_kernel_spmd(nc, in_maps, list(range(N_CORES)), trace=trace)
    q = np.stack(
        [res.results[c]["out"].reshape(B_LOC, 2) for c in range(N_CORES)]
    ).astype(np.float64)
    s1 = -q[..., 0].sum()
    s2 = -q[..., 1].sum()
    sum_obj = float(np.asarray(objectness_label, np.float64).sum())
    sum_mask = float(np.asarray(box_label_mask, np.float64).sum())
    loss = s1 / (sum_obj + 1e-6) + s2 / (sum_mask + 1e-6)
    return np.float32(loss), res


def kernel(pred_center, center_label, box_label_mask, objectness_label):
    loss, _ = run(pred_center, center_label, box_label_mask, objectness_label)
    return np.array(loss, dtype=np.float32)
